# revision 10
# baseline (speedup 1.0000x reference)
"""Multi-head attention forward (B=16, S=1024, d=1024, H=16, Dh=64) on 8
Trainium2 NeuronCores, data-parallel over batch (2 batches per core).

v2: row-tiled concurrent scores pairs (K=64 in rows 0-63 / 64-127), N=1024
cross-bank exp activations, deadline-ordered projection work queue to keep
the PE queue free of head-of-line stalls.

Device kernel (per core, bf16 matmuls, fp32 accumulate):
  inputs (host-prepped): XT [d, 2048] = hidden[2c:2c+2].reshape(2048,d).T,
  WqT/WkT/WvT = W.T [in, out], WoT = Wo.T [dv, o]  (all bf16),
  bq, bk [1024] f32, bo2 = bo + Wo @ bv  (bv folded: softmax rows sum to 1).

  Per (b, j=head-pair): QT/KT [128, 1024] (pair dims on partitions,
  h0 dims 0-63, h1 dims 64-127).
  scores pair (per st key-chunk, per c query-half):
    sc[:, 0:512]    = KT[0:64, st].T  @ QT[0:64, c]    (PE rows 0-63)
    sc[:, 512:1024] = KT[64:128, st].T @ QT[64:128, c] (PE rows 64-127, concurrent)
  exp over [128, 1024] psum (2 banks) -> pt[st] bf16.
  PV: ctx_aug[65, 1024] += [V_h | 1].T @ P_h  (row 64 = softmax denominator)
  normalize: ctx * bcast(1/denom); h1-half staged + DMA partition shift.
  outT[o, t] = WoT.T @ ctxT (+bo2).
"""

from collections import deque

import numpy as np
import ml_dtypes

import concourse.bass as bass
import concourse.mybir as mybir
import concourse.tile as tile
from concourse import bacc
from concourse.bass_utils import run_bass_kernel_spmd

P = 128
D = 1024
T = 2048  # tokens per core
TB = 1024  # tokens per batch (= S)
H = 16
DH = 64
KD = D // P  # 8 chunks of the d/dv/s dims
NB = T // TB  # batches per core
NCORES = 8

BF16 = mybir.dt.bfloat16
F32 = mybir.dt.float32
EXPF = mybir.ActivationFunctionType.Exp
MULT = mybir.AluOpType.mult

# test.py hooks
TRACE = False
TRACE_KWARGS = {}
LAST_RESULTS = None

_NC_CACHE = None


def build_nc():
    nc = bacc.Bacc("TRN2", target_bir_lowering=False, debug=False, num_devices=NCORES)

    xt_d = nc.dram_tensor("xt", [D, T], BF16, kind="ExternalInput")
    wqt_d = nc.dram_tensor("wqt", [D, D], BF16, kind="ExternalInput")
    wkt_d = nc.dram_tensor("wkt", [D, D], BF16, kind="ExternalInput")
    wvt_d = nc.dram_tensor("wvt", [D, D], BF16, kind="ExternalInput")
    wot_d = nc.dram_tensor("wot", [D, D], BF16, kind="ExternalInput")
    bq_d = nc.dram_tensor("bq", [D], F32, kind="ExternalInput")
    bk_d = nc.dram_tensor("bk", [D], F32, kind="ExternalInput")
    bo2_d = nc.dram_tensor("bo2", [D], F32, kind="ExternalInput")
    outt_d = nc.dram_tensor("outt", [D, T], F32, kind="ExternalOutput")

    with tile.TileContext(nc) as tc:
        from contextlib import ExitStack

        with ExitStack() as ctx:
            wpool = ctx.enter_context(tc.tile_pool(name="w", bufs=1))
            xpool = ctx.enter_context(tc.tile_pool(name="x", bufs=1))
            spool = ctx.enter_context(tc.tile_pool(name="small", bufs=1))
            qkpool = ctx.enter_context(tc.tile_pool(name="qk", bufs=2))
            vpool = ctx.enter_context(tc.tile_pool(name="v", bufs=2))
            ptpool = ctx.enter_context(tc.tile_pool(name="pt", bufs=3))
            cpool = ctx.enter_context(tc.tile_pool(name="ctx", bufs=2))
            npool = ctx.enter_context(tc.tile_pool(name="norm", bufs=2))
            opool = ctx.enter_context(tc.tile_pool(name="out", bufs=2))
            scp = ctx.enter_context(tc.tile_pool(name="scp", bufs=1, space="PSUM"))
            pvp = ctx.enter_context(tc.tile_pool(name="pvp", bufs=1, space="PSUM"))
            accp = ctx.enter_context(tc.tile_pool(name="accp", bufs=2, space="PSUM"))

            # ---- global loads ----
            xt = [xpool.tile([P, T], BF16, tag=f"xt{k}", name=f"xt{k}") for k in range(KD)]
            wq, wk, wv, wo = (
                [wpool.tile([P, D], BF16, tag=f"w{nm}{k}", name=f"w{nm}{k}") for k in range(KD)]
                for nm in "qkvo"
            )
            for k in range(KD):
                nc.sync.dma_start(wv[k][:], wvt_d[k * P : (k + 1) * P, :])
                nc.sync.dma_start(xt[k][:], xt_d[k * P : (k + 1) * P, :])
            for wt, wd in ((wq, wqt_d), (wk, wkt_d), (wo, wot_d)):
                for k in range(KD):
                    nc.sync.dma_start(wt[k][:], wd[k * P : (k + 1) * P, :])
            bq_sb = spool.tile([P, KD], F32, tag="bq", name="bq_sb")
            bk_sb = spool.tile([P, KD], F32, tag="bk", name="bk_sb")
            bo_sb = spool.tile([P, KD], F32, tag="bo", name="bo_sb")
            for sb, dram in ((bq_sb, bq_d), (bk_sb, bk_d), (bo_sb, bo2_d)):
                nc.sync.dma_start(sb[:], dram.rearrange("(o p) -> p o", p=P))

            # ---- work queue of projection thunks (1 instruction each) ----
            accq = deque()

            def pump(n):
                for _ in range(n):
                    if not accq:
                        return
                    accq.popleft()()

            def qk_group(get_dst, wt, bias_sb, j, b, c):
                """8 MMs accumulating [128,512] + bias drain into dst."""
                st8 = {}

                def mm(k):
                    if "ps" not in st8:
                        st8["ps"] = accp.tile([P, 512], F32, tag="acc", name="acc")
                    nc.tensor.matmul(
                        st8["ps"][:],
                        wt[k][:, j * P : (j + 1) * P],
                        xt[k][:, b * TB + c * 512 : b * TB + (c + 1) * 512],
                        start=(k == 0),
                        stop=(k == KD - 1),
                    )

                def drain():
                    nc.vector.tensor_scalar_add(
                        get_dst()[:, c * 512 : (c + 1) * 512],
                        st8["ps"][:],
                        bias_sb[:, j : j + 1],
                    )

                return [lambda k=k: mm(k) for k in range(KD)] + [drain]

            def v_group(get_vt, b, mt, c):
                st8 = {}

                def mm(k):
                    if "ps" not in st8:
                        st8["ps"] = accp.tile([P, 512], F32, tag="acc", name="acc")
                    nc.tensor.matmul(
                        st8["ps"][:],
                        xt[k][:, (b * KD + mt) * P : (b * KD + mt + 1) * P],
                        wv[k][:, c * 512 : (c + 1) * 512],
                        start=(k == 0),
                        stop=(k == KD - 1),
                    )

                def drain():
                    nc.vector.tensor_copy(
                        get_vt()[:, c * 8 : (c + 1) * 8, 0:DH],
                        st8["ps"].rearrange("p (h d) -> p h d", d=DH),
                    )

                return [lambda k=k: mm(k) for k in range(KD)] + [drain]

            def o_group(ctxt, b, mo, c):
                st8 = {}

                def mm(k):
                    if "ps" not in st8:
                        st8["ps"] = accp.tile([P, 512], F32, tag="acc", name="acc")
                    nc.tensor.matmul(
                        st8["ps"][:],
                        wo[k][:, mo * P : (mo + 1) * P],
                        ctxt[k][:, c * 512 : (c + 1) * 512],
                        start=(k == 0),
                        stop=(k == KD - 1),
                    )

                def drain():
                    osb = opool.tile([P, 512], F32, tag="osb", name="osb")
                    nc.vector.tensor_scalar_add(osb[:], st8["ps"][:], bo_sb[:, mo : mo + 1])
                    nc.sync.dma_start(
                        outt_d[
                            mo * P : (mo + 1) * P,
                            b * TB + c * 512 : b * TB + (c + 1) * 512,
                        ],
                        osb[:],
                    )

                return [lambda k=k: mm(k) for k in range(KD)] + [drain]

            # ---- tile registries (lazily created inside thunks) ----
            qk_tiles = {}  # (j, b, 'q'|'k') -> tile [P, TB]

            def get_qk(j, b, which):
                key = (j, b, which)
                if key not in qk_tiles:
                    qk_tiles[key] = qkpool.tile(
                        [P, TB], BF16, tag=f"{which}tj", name=f"{which}t{j}b{b}"
                    )
                return qk_tiles[key]

            v_tiles = {}  # (b, mt) -> tile [P, H, DH+1]

            def get_v(b, mt):
                key = (b, mt)
                if key not in v_tiles:
                    t = vpool.tile([P, H, DH + 1], BF16, tag=f"v{mt}", name=f"v{mt}b{b}")
                    v_tiles[key] = t
                    nc.vector.memset(t[:, :, DH : DH + 1], 1.0)
                return v_tiles[key]

            ctxt_tiles = {}  # b -> [8 tiles]

            def get_ctxt(b):
                if b not in ctxt_tiles:
                    ctxt_tiles[b] = [
                        cpool.tile([P, TB], BF16, tag=f"ctxt{m}", name=f"ctxt{m}b{b}")
                        for m in range(KD)
                    ]
                return ctxt_tiles[b]

            def enqueue_qk(j, b):
                for which, wt, bias in (("q", wq, bq_sb), ("k", wk, bk_sb)):
                    for c in range(2):
                        accq.extend(
                            qk_group(lambda j=j, b=b, w=which: get_qk(j, b, w), wt, bias, j, b, c)
                        )

            def enqueue_v(b, mts):
                for mt in mts:
                    for c in range(2):
                        accq.extend(v_group(lambda b=b, mt=mt: get_v(b, mt), b, mt, c))

            def enqueue_o(b, mos):
                ctxt = get_ctxt(b)
                for mo in mos:
                    for c in range(2):
                        accq.extend(o_group(ctxt, b, mo, c))

            # ---- attention slot for (j, b) ----
            def attention_slot(j, b, pump_n=4):
                qtj = get_qk(j, b, "q")
                ktj = get_qk(j, b, "k")
                ctxt = get_ctxt(b)
                pv0 = pvp.tile([P, TB], F32, tag="pv0", name=f"pv0_{j}_{b}")
                pv1 = pvp.tile([P, TB], F32, tag="pv1", name=f"pv1_{j}_{b}")
                h0, h1 = 2 * j, 2 * j + 1
                pts = []

                def pv_half(st, hh):
                    pt = pts[st]
                    vt = get_v(b, st)
                    first, last = st == 0, st == KD - 1
                    pv = pv0 if hh == 0 else pv1
                    h = h0 if hh == 0 else h1
                    off = 0 if hh == 0 else 512
                    nc.tensor.matmul(
                        pv[0 : DH + 1, 0:512], vt[:, h, :], pt[:, off : off + 512],
                        start=first, stop=last,
                    )
                    nc.tensor.matmul(
                        pv[0 : DH + 1, 512:1024], vt[:, h, :],
                        pt[:, 1024 + off : 1536 + off],
                        start=first, stop=last,
                    )

                for st in range(KD):
                    pt = ptpool.tile([P, 2048], BF16, tag="pt", name=f"pt{st}")
                    pts.append(pt)
                    for cg in range(2):
                        sc = scp.tile([P, 1024], F32, tag="sc", name="sc")
                        nc.tensor.matmul(
                            sc[:, 0:512],
                            ktj[0:DH, st * P : (st + 1) * P],
                            qtj[0:DH, cg * 512 : (cg + 1) * 512],
                            start=True, stop=True,
                        )
                        nc.tensor.matmul(
                            sc[:, 512:1024],
                            ktj[DH:P, st * P : (st + 1) * P],
                            qtj[DH:P, cg * 512 : (cg + 1) * 512],
                            start=True, stop=True,
                        )
                        nc.scalar.activation(
                            pt[:, cg * 1024 : (cg + 1) * 1024],
                            sc[:, 0:1024],
                            EXPF,
                            scale=0.125,
                        )
                        if st > 0:
                            pv_half(st - 1, cg)
                        pump(pump_n)
                pv_half(KD - 1, 0)
                pv_half(KD - 1, 1)

                # normalize h0 -> ctxt rows 0:64, h1 -> staged + DMA shift
                for hh, pv in ((0, pv0), (1, pv1)):
                    rs = npool.tile([1, TB], F32, tag="rs", name="rs", bufs=1)
                    nc.vector.tensor_copy(rs[:], pv[DH : DH + 1, :])
                    rr = npool.tile([1, TB], F32, tag="rr", name="rr", bufs=1)
                    nc.vector.reciprocal_approx_fast(rr[:], rs[:])
                    rb = npool.tile([DH, TB], F32, tag="rb", name="rb")
                    nc.gpsimd.partition_broadcast(rb[:], rr[:])
                    if hh == 0:
                        nc.vector.tensor_tensor(
                            ctxt[j][0:DH, :], pv[0:DH, :], rb[:], MULT
                        )
                    else:
                        ch = npool.tile([DH, TB], BF16, tag="ch", name="ch")
                        nc.vector.tensor_tensor(ch[:], pv[0:DH, :], rb[:], MULT)
                        nc.sync.dma_start(ctxt[j][DH:P, :], ch[:])

            # ---- schedule ----
            # prologue: QK(j0, b0) + V(b0) drained
            enqueue_qk(0, 0)
            enqueue_v(0, range(KD))
            pump(len(accq))

            # batch 0 slots; V(b1) spread over slots 2-5, QK(j0,b1) at slot 6
            for j in range(KD):
                if j + 1 < KD:
                    enqueue_qk(j + 1, 0)
                if 2 <= j <= 5:
                    enqueue_v(1, range((j - 2) * 2, (j - 2) * 2 + 2))
                if j == 6:
                    enqueue_qk(0, 1)
                attention_slot(j, 0, pump_n=10 if j == 0 else 4)

            # batch 1 slots; o-proj of b0 spread two mo per early slot
            for j in range(KD):
                if j + 1 < KD:
                    enqueue_qk(j + 1, 1)
                if j < 4:
                    enqueue_o(0, [2 * j, 2 * j + 1])
                attention_slot(j, 1)
            enqueue_o(1, range(KD))
            pump(len(accq))

    nc.compile()
    return nc


def _get_nc():
    global _NC_CACHE
    if _NC_CACHE is None:
        _NC_CACHE = build_nc()
    return _NC_CACHE


def kernel(hidden_states, Wq, bq, Wk, bk, Wv, bv, Wo, bo):
    global LAST_RESULTS
    bf = ml_dtypes.bfloat16
    hs = np.asarray(hidden_states, np.float32)
    Wq = np.asarray(Wq, np.float32)
    Wk = np.asarray(Wk, np.float32)
    Wv = np.asarray(Wv, np.float32)
    Wo = np.asarray(Wo, np.float32)
    bq = np.asarray(bq, np.float32)
    bk = np.asarray(bk, np.float32)
    bv = np.asarray(bv, np.float32)
    bo = np.asarray(bo, np.float32)

    wqt = np.ascontiguousarray(Wq.T).astype(bf)
    wkt = np.ascontiguousarray(Wk.T).astype(bf)
    wvt = np.ascontiguousarray(Wv.T).astype(bf)
    wot = np.ascontiguousarray(Wo.T).astype(bf)
    bo2 = (bo + Wo @ bv).astype(np.float32)

    bpc = hs.shape[0] // NCORES  # batches per core
    in_maps = []
    for c in range(NCORES):
        xc = hs[c * bpc : (c + 1) * bpc].reshape(bpc * TB, D)
        in_maps.append(
            {
                "xt": np.ascontiguousarray(xc.T).astype(bf),
                "wqt": wqt,
                "wkt": wkt,
                "wvt": wvt,
                "wot": wot,
                "bq": bq,
                "bk": bk,
                "bo2": bo2,
            }
        )

    nc = _get_nc()
    res = run_bass_kernel_spmd(
        nc,
        in_maps,
        core_ids=list(range(NCORES)),
        trace=TRACE,
        **TRACE_KWARGS,
    )
    LAST_RESULTS = res

    out = np.empty((hs.shape[0], TB, D), np.float32)
    for c in range(NCORES):
        ot = res.results[c]["outt"]  # [D, T]
        for b in range(bpc):
            out[c * bpc + b] = ot[:, b * TB : (b + 1) * TB].T
    return out


# revision 12
# speedup vs baseline: 1.0486x; 1.0486x over previous
"""Multi-head attention forward (B=16, S=1024, d=1024, H=16, Dh=64) on 8
Trainium2 NeuronCores, data-parallel over batch (2 batches per core).

v2: row-tiled concurrent scores pairs (K=64 in rows 0-63 / 64-127), N=1024
cross-bank exp activations, deadline-ordered projection work queue to keep
the PE queue free of head-of-line stalls.

Device kernel (per core, bf16 matmuls, fp32 accumulate):
  inputs (host-prepped): XT [d, 2048] = hidden[2c:2c+2].reshape(2048,d).T,
  WqT/WkT/WvT = W.T [in, out], WoT = Wo.T [dv, o]  (all bf16),
  bq, bk [1024] f32, bo2 = bo + Wo @ bv  (bv folded: softmax rows sum to 1).

  Per (b, j=head-pair): QT/KT [128, 1024] (pair dims on partitions,
  h0 dims 0-63, h1 dims 64-127).
  scores pair (per st key-chunk, per c query-half):
    sc[:, 0:512]    = KT[0:64, st].T  @ QT[0:64, c]    (PE rows 0-63)
    sc[:, 512:1024] = KT[64:128, st].T @ QT[64:128, c] (PE rows 64-127, concurrent)
  exp over [128, 1024] psum (2 banks) -> pt[st] bf16.
  PV: ctx_aug[65, 1024] += [V_h | 1].T @ P_h  (row 64 = softmax denominator)
  normalize: ctx * bcast(1/denom); h1-half staged + DMA partition shift.
  outT[o, t] = WoT.T @ ctxT (+bo2).
"""

from collections import deque

import numpy as np
import ml_dtypes

import concourse.bass as bass
import concourse.mybir as mybir
import concourse.tile as tile
from concourse import bacc
from concourse.bass_utils import run_bass_kernel_spmd

P = 128
D = 1024
T = 2048  # tokens per core
TB = 1024  # tokens per batch (= S)
H = 16
DH = 64
KD = D // P  # 8 chunks of the d/dv/s dims
NB = T // TB  # batches per core
NCORES = 8

BF16 = mybir.dt.bfloat16
F32 = mybir.dt.float32
EXPF = mybir.ActivationFunctionType.Exp
MULT = mybir.AluOpType.mult

# test.py hooks
TRACE = False
TRACE_KWARGS = {}
LAST_RESULTS = None

_NC_CACHE = None


def build_nc():
    nc = bacc.Bacc("TRN2", target_bir_lowering=False, debug=False, num_devices=NCORES)

    xt_d = nc.dram_tensor("xt", [D, T], BF16, kind="ExternalInput")
    wqt_d = nc.dram_tensor("wqt", [D, D], BF16, kind="ExternalInput")
    wkt_d = nc.dram_tensor("wkt", [D, D], BF16, kind="ExternalInput")
    wvt_d = nc.dram_tensor("wvt", [D, D], BF16, kind="ExternalInput")
    wot_d = nc.dram_tensor("wot", [D, D], BF16, kind="ExternalInput")
    bq_d = nc.dram_tensor("bq", [D], F32, kind="ExternalInput")
    bk_d = nc.dram_tensor("bk", [D], F32, kind="ExternalInput")
    bo2_d = nc.dram_tensor("bo2", [D], F32, kind="ExternalInput")
    outt_d = nc.dram_tensor("outt", [D, T], F32, kind="ExternalOutput")

    with tile.TileContext(nc) as tc:
        from contextlib import ExitStack

        with ExitStack() as ctx:
            wpool = ctx.enter_context(tc.tile_pool(name="w", bufs=1))
            xpool = ctx.enter_context(tc.tile_pool(name="x", bufs=1))
            spool = ctx.enter_context(tc.tile_pool(name="small", bufs=1))
            qkpool = ctx.enter_context(tc.tile_pool(name="qk", bufs=2))
            vpool = ctx.enter_context(tc.tile_pool(name="v", bufs=2))
            ptpool = ctx.enter_context(tc.tile_pool(name="pt", bufs=3))
            cpool = ctx.enter_context(tc.tile_pool(name="ctx", bufs=2))
            npool = ctx.enter_context(tc.tile_pool(name="norm", bufs=2))
            opool = ctx.enter_context(tc.tile_pool(name="out", bufs=2))
            scp = ctx.enter_context(tc.tile_pool(name="scp", bufs=2, space="PSUM"))
            pvp = ctx.enter_context(tc.tile_pool(name="pvp", bufs=1, space="PSUM"))
            accp = ctx.enter_context(tc.tile_pool(name="accp", bufs=2, space="PSUM"))

            # ---- global loads ----
            xt = [xpool.tile([P, T], BF16, tag=f"xt{k}", name=f"xt{k}") for k in range(KD)]
            wq, wk, wv, wo = (
                [wpool.tile([P, D], BF16, tag=f"w{nm}{k}", name=f"w{nm}{k}") for k in range(KD)]
                for nm in "qkvo"
            )
            for k in range(KD):
                nc.sync.dma_start(wv[k][:], wvt_d[k * P : (k + 1) * P, :])
                nc.sync.dma_start(xt[k][:], xt_d[k * P : (k + 1) * P, :])
            for wt, wd in ((wq, wqt_d), (wk, wkt_d), (wo, wot_d)):
                for k in range(KD):
                    nc.sync.dma_start(wt[k][:], wd[k * P : (k + 1) * P, :])
            bq_sb = spool.tile([P, KD], F32, tag="bq", name="bq_sb")
            bk_sb = spool.tile([P, KD], F32, tag="bk", name="bk_sb")
            bo_sb = spool.tile([P, KD], F32, tag="bo", name="bo_sb")
            for sb, dram in ((bq_sb, bq_d), (bk_sb, bk_d), (bo_sb, bo2_d)):
                nc.sync.dma_start(sb[:], dram.rearrange("(o p) -> p o", p=P))

            # ---- work queue of projection thunks (1 instruction each) ----
            accq = deque()

            def pump(n):
                for _ in range(n):
                    if not accq:
                        return
                    accq.popleft()()

            def qk_group(get_dst, wt, bias_sb, j, b, c):
                """8 MMs accumulating [128,512] + bias drain into dst."""
                st8 = {}

                def mm(k):
                    if "ps" not in st8:
                        st8["ps"] = accp.tile([P, 512], F32, tag="acc", name="acc")
                    nc.tensor.matmul(
                        st8["ps"][:],
                        wt[k][:, j * P : (j + 1) * P],
                        xt[k][:, b * TB + c * 512 : b * TB + (c + 1) * 512],
                        start=(k == 0),
                        stop=(k == KD - 1),
                    )

                def drain():
                    nc.vector.tensor_scalar_add(
                        get_dst()[:, c * 512 : (c + 1) * 512],
                        st8["ps"][:],
                        bias_sb[:, j : j + 1],
                    )

                return [lambda k=k: mm(k) for k in range(KD)] + [drain]

            def v_group(get_vt, b, mt, c):
                st8 = {}

                def mm(k):
                    if "ps" not in st8:
                        st8["ps"] = accp.tile([P, 512], F32, tag="acc", name="acc")
                    nc.tensor.matmul(
                        st8["ps"][:],
                        xt[k][:, (b * KD + mt) * P : (b * KD + mt + 1) * P],
                        wv[k][:, c * 512 : (c + 1) * 512],
                        start=(k == 0),
                        stop=(k == KD - 1),
                    )

                def drain():
                    nc.vector.tensor_copy(
                        get_vt()[:, c * 8 : (c + 1) * 8, 0:DH],
                        st8["ps"].rearrange("p (h d) -> p h d", d=DH),
                    )

                return [lambda k=k: mm(k) for k in range(KD)] + [drain]

            def o_group(ctxt, b, mo, c):
                st8 = {}

                def mm(k):
                    if "ps" not in st8:
                        st8["ps"] = accp.tile([P, 512], F32, tag="acc", name="acc")
                    nc.tensor.matmul(
                        st8["ps"][:],
                        wo[k][:, mo * P : (mo + 1) * P],
                        ctxt[k][:, c * 512 : (c + 1) * 512],
                        start=(k == 0),
                        stop=(k == KD - 1),
                    )

                def drain():
                    osb = opool.tile([P, 512], F32, tag="osb", name="osb")
                    nc.vector.tensor_scalar_add(osb[:], st8["ps"][:], bo_sb[:, mo : mo + 1])
                    nc.sync.dma_start(
                        outt_d[
                            mo * P : (mo + 1) * P,
                            b * TB + c * 512 : b * TB + (c + 1) * 512,
                        ],
                        osb[:],
                    )

                return [lambda k=k: mm(k) for k in range(KD)] + [drain]

            # ---- tile registries (lazily created inside thunks) ----
            qk_tiles = {}  # (j, b, 'q'|'k') -> tile [P, TB]

            def get_qk(j, b, which):
                key = (j, b, which)
                if key not in qk_tiles:
                    qk_tiles[key] = qkpool.tile(
                        [P, TB], BF16, tag=f"{which}tj", name=f"{which}t{j}b{b}"
                    )
                return qk_tiles[key]

            v_tiles = {}  # (b, mt) -> tile [P, H, DH+1]

            def get_v(b, mt):
                key = (b, mt)
                if key not in v_tiles:
                    t = vpool.tile([P, H, DH + 1], BF16, tag=f"v{mt}", name=f"v{mt}b{b}")
                    v_tiles[key] = t
                    nc.vector.memset(t[:, :, DH : DH + 1], 1.0)
                return v_tiles[key]

            ctxt_tiles = {}  # b -> [8 tiles]

            def get_ctxt(b):
                if b not in ctxt_tiles:
                    ctxt_tiles[b] = [
                        cpool.tile([P, TB], BF16, tag=f"ctxt{m}", name=f"ctxt{m}b{b}")
                        for m in range(KD)
                    ]
                return ctxt_tiles[b]

            def enqueue_qk(j, b):
                for which, wt, bias in (("q", wq, bq_sb), ("k", wk, bk_sb)):
                    for c in range(2):
                        accq.extend(
                            qk_group(lambda j=j, b=b, w=which: get_qk(j, b, w), wt, bias, j, b, c)
                        )

            def enqueue_v(b, mts):
                for mt in mts:
                    for c in range(2):
                        accq.extend(v_group(lambda b=b, mt=mt: get_v(b, mt), b, mt, c))

            def enqueue_o(b, mos):
                ctxt = get_ctxt(b)
                for mo in mos:
                    for c in range(2):
                        accq.extend(o_group(ctxt, b, mo, c))

            # ---- attention slot for (j, b) ----
            def attention_slot(j, b, pump_n=4):
                qtj = get_qk(j, b, "q")
                ktj = get_qk(j, b, "k")
                ctxt = get_ctxt(b)
                h0, h1 = 2 * j, 2 * j + 1

                for cg in range(2):  # query half: 512 tokens per pass
                    q0, q1 = cg * 512, (cg + 1) * 512
                    pv0 = pvp.tile([P, 512], F32, tag="pv0", name=f"pv0_{j}_{b}_{cg}")
                    pv1 = pvp.tile([P, 512], F32, tag="pv1", name=f"pv1_{j}_{b}_{cg}")
                    pts = []

                    def pv_step(st, pv0=pv0, pv1=pv1, pts=pts):
                        pt = pts[st]
                        vt = get_v(b, st)
                        first, last = st == 0, st == KD - 1
                        nc.tensor.matmul(
                            pv0[0 : DH + 1, :], vt[:, h0, :], pt[:, 0:512],
                            start=first, stop=last,
                        )
                        nc.tensor.matmul(
                            pv1[0 : DH + 1, :], vt[:, h1, :], pt[:, 512:1024],
                            start=first, stop=last,
                        )

                    for st in range(KD):
                        pt = ptpool.tile([P, 1024], BF16, tag="pt", name=f"pt{st}")
                        pts.append(pt)
                        sc = scp.tile([P, 1024], F32, tag="sc", name="sc")
                        nc.tensor.matmul(
                            sc[:, 0:512],
                            ktj[0:DH, st * P : (st + 1) * P],
                            qtj[0:DH, q0:q1],
                            start=True, stop=True,
                        )
                        nc.tensor.matmul(
                            sc[:, 512:1024],
                            ktj[DH:P, st * P : (st + 1) * P],
                            qtj[DH:P, q0:q1],
                            start=True, stop=True,
                        )
                        nc.scalar.activation(
                            pt[:, 0:1024], sc[:, 0:1024], EXPF, scale=0.125
                        )
                        if st > 0:
                            pv_step(st - 1)
                        pump(pump_n)
                    pv_step(KD - 1)

                    # normalize h0 -> ctxt rows 0:64, h1 -> staged + DMA shift
                    for hh, pv in ((0, pv0), (1, pv1)):
                        rs = npool.tile([1, 512], F32, tag="rs", name="rs", bufs=1)
                        nc.vector.tensor_copy(rs[:], pv[DH : DH + 1, :])
                        rr = npool.tile([1, 512], F32, tag="rr", name="rr", bufs=1)
                        nc.vector.reciprocal_approx_fast(rr[:], rs[:])
                        rb = npool.tile([DH, 512], F32, tag="rb", name="rb")
                        nc.gpsimd.partition_broadcast(rb[:], rr[:])
                        if hh == 0:
                            nc.vector.tensor_tensor(
                                ctxt[j][0:DH, q0:q1], pv[0:DH, :], rb[:], MULT
                            )
                        else:
                            ch = npool.tile([DH, 512], BF16, tag="ch", name="ch")
                            nc.vector.tensor_tensor(ch[:], pv[0:DH, :], rb[:], MULT)
                            nc.sync.dma_start(ctxt[j][DH:P, q0:q1], ch[:])

            # ---- schedule ----
            # prologue: QK(j0, b0) + V(b0) drained
            enqueue_qk(0, 0)
            enqueue_v(0, range(KD))
            pump(len(accq))

            # batch 0 slots; V(b1) spread over slots 2-5, QK(j0,b1) at slot 6
            for j in range(KD):
                if j + 1 < KD:
                    enqueue_qk(j + 1, 0)
                if 2 <= j <= 5:
                    enqueue_v(1, range((j - 2) * 2, (j - 2) * 2 + 2))
                if j == 6:
                    enqueue_qk(0, 1)
                attention_slot(j, 0, pump_n=10 if j == 0 else 4)

            # batch 1 slots; o-proj of b0 spread two mo per early slot
            for j in range(KD):
                if j + 1 < KD:
                    enqueue_qk(j + 1, 1)
                if j < 4:
                    enqueue_o(0, [2 * j, 2 * j + 1])
                attention_slot(j, 1)
            enqueue_o(1, range(KD))
            pump(len(accq))

    nc.compile()
    return nc


def _get_nc():
    global _NC_CACHE
    if _NC_CACHE is None:
        _NC_CACHE = build_nc()
    return _NC_CACHE


def kernel(hidden_states, Wq, bq, Wk, bk, Wv, bv, Wo, bo):
    global LAST_RESULTS
    bf = ml_dtypes.bfloat16
    hs = np.asarray(hidden_states, np.float32)
    Wq = np.asarray(Wq, np.float32)
    Wk = np.asarray(Wk, np.float32)
    Wv = np.asarray(Wv, np.float32)
    Wo = np.asarray(Wo, np.float32)
    bq = np.asarray(bq, np.float32)
    bk = np.asarray(bk, np.float32)
    bv = np.asarray(bv, np.float32)
    bo = np.asarray(bo, np.float32)

    wqt = np.ascontiguousarray(Wq.T).astype(bf)
    wkt = np.ascontiguousarray(Wk.T).astype(bf)
    wvt = np.ascontiguousarray(Wv.T).astype(bf)
    wot = np.ascontiguousarray(Wo.T).astype(bf)
    bo2 = (bo + Wo @ bv).astype(np.float32)

    bpc = hs.shape[0] // NCORES  # batches per core
    in_maps = []
    for c in range(NCORES):
        xc = hs[c * bpc : (c + 1) * bpc].reshape(bpc * TB, D)
        in_maps.append(
            {
                "xt": np.ascontiguousarray(xc.T).astype(bf),
                "wqt": wqt,
                "wkt": wkt,
                "wvt": wvt,
                "wot": wot,
                "bq": bq,
                "bk": bk,
                "bo2": bo2,
            }
        )

    nc = _get_nc()
    res = run_bass_kernel_spmd(
        nc,
        in_maps,
        core_ids=list(range(NCORES)),
        trace=TRACE,
        **TRACE_KWARGS,
    )
    LAST_RESULTS = res

    out = np.empty((hs.shape[0], TB, D), np.float32)
    for c in range(NCORES):
        ot = res.results[c]["outt"]  # [D, T]
        for b in range(bpc):
            out[c * bpc + b] = ot[:, b * TB : (b + 1) * TB].T
    return out


# revision 13
# speedup vs baseline: 1.0802x; 1.0302x over previous
"""Multi-head attention forward (B=16, S=1024, d=1024, H=16, Dh=64) on 8
Trainium2 NeuronCores, data-parallel over batch (2 batches per core).

v2: row-tiled concurrent scores pairs (K=64 in rows 0-63 / 64-127), N=1024
cross-bank exp activations, deadline-ordered projection work queue to keep
the PE queue free of head-of-line stalls.

Device kernel (per core, bf16 matmuls, fp32 accumulate):
  inputs (host-prepped): XT [d, 2048] = hidden[2c:2c+2].reshape(2048,d).T,
  WqT/WkT/WvT = W.T [in, out], WoT = Wo.T [dv, o]  (all bf16),
  bq, bk [1024] f32, bo2 = bo + Wo @ bv  (bv folded: softmax rows sum to 1).

  Per (b, j=head-pair): QT/KT [128, 1024] (pair dims on partitions,
  h0 dims 0-63, h1 dims 64-127).
  scores pair (per st key-chunk, per c query-half):
    sc[:, 0:512]    = KT[0:64, st].T  @ QT[0:64, c]    (PE rows 0-63)
    sc[:, 512:1024] = KT[64:128, st].T @ QT[64:128, c] (PE rows 64-127, concurrent)
  exp over [128, 1024] psum (2 banks) -> pt[st] bf16.
  PV: ctx_aug[65, 1024] += [V_h | 1].T @ P_h  (row 64 = softmax denominator)
  normalize: ctx * bcast(1/denom); h1-half staged + DMA partition shift.
  outT[o, t] = WoT.T @ ctxT (+bo2).
"""

from collections import deque

import numpy as np
import ml_dtypes

import concourse.bass as bass
import concourse.mybir as mybir
import concourse.tile as tile
from concourse import bacc
from concourse.bass_utils import run_bass_kernel_spmd

P = 128
D = 1024
T = 2048  # tokens per core
TB = 1024  # tokens per batch (= S)
H = 16
DH = 64
KD = D // P  # 8 chunks of the d/dv/s dims
NB = T // TB  # batches per core
NCORES = 8

BF16 = mybir.dt.bfloat16
F32 = mybir.dt.float32
EXPF = mybir.ActivationFunctionType.Exp
IDF = mybir.ActivationFunctionType.Identity
MULT = mybir.AluOpType.mult

# test.py hooks
TRACE = False
TRACE_KWARGS = {}
LAST_RESULTS = None

_NC_CACHE = None


def build_nc():
    nc = bacc.Bacc("TRN2", target_bir_lowering=False, debug=False, num_devices=NCORES)

    xt_d = nc.dram_tensor("xt", [D, T], BF16, kind="ExternalInput")
    wqt_d = nc.dram_tensor("wqt", [D, D], BF16, kind="ExternalInput")
    wkt_d = nc.dram_tensor("wkt", [D, D], BF16, kind="ExternalInput")
    wvt_d = nc.dram_tensor("wvt", [D, D], BF16, kind="ExternalInput")
    wot_d = nc.dram_tensor("wot", [D, D], BF16, kind="ExternalInput")
    bq_d = nc.dram_tensor("bq", [D], F32, kind="ExternalInput")
    bk_d = nc.dram_tensor("bk", [D], F32, kind="ExternalInput")
    bo2_d = nc.dram_tensor("bo2", [D], F32, kind="ExternalInput")
    outt_d = nc.dram_tensor("outt", [D, T], F32, kind="ExternalOutput")

    with tile.TileContext(nc) as tc:
        from contextlib import ExitStack

        with ExitStack() as ctx:
            wpool = ctx.enter_context(tc.tile_pool(name="w", bufs=1))
            xpool = ctx.enter_context(tc.tile_pool(name="x", bufs=1))
            spool = ctx.enter_context(tc.tile_pool(name="small", bufs=1))
            qkpool = ctx.enter_context(tc.tile_pool(name="qk", bufs=2))
            vpool = ctx.enter_context(tc.tile_pool(name="v", bufs=2))
            ptpool = ctx.enter_context(tc.tile_pool(name="pt", bufs=6))
            cpool = ctx.enter_context(tc.tile_pool(name="ctx", bufs=2))
            npool = ctx.enter_context(tc.tile_pool(name="norm", bufs=2))
            opool = ctx.enter_context(tc.tile_pool(name="out", bufs=2))
            scp = ctx.enter_context(tc.tile_pool(name="scp", bufs=2, space="PSUM"))
            pvp = ctx.enter_context(tc.tile_pool(name="pvp", bufs=1, space="PSUM"))
            accp = ctx.enter_context(tc.tile_pool(name="accp", bufs=2, space="PSUM"))

            # ---- global loads ----
            xt = [xpool.tile([P, T], BF16, tag=f"xt{k}", name=f"xt{k}") for k in range(KD)]
            wq, wk, wv, wo = (
                [wpool.tile([P, D], BF16, tag=f"w{nm}{k}", name=f"w{nm}{k}") for k in range(KD)]
                for nm in "qkvo"
            )
            for k in range(KD):
                nc.sync.dma_start(wv[k][:], wvt_d[k * P : (k + 1) * P, :])
                nc.sync.dma_start(xt[k][:], xt_d[k * P : (k + 1) * P, :])
            for wt, wd in ((wq, wqt_d), (wk, wkt_d), (wo, wot_d)):
                for k in range(KD):
                    nc.sync.dma_start(wt[k][:], wd[k * P : (k + 1) * P, :])
            bq_sb = spool.tile([P, KD], F32, tag="bq", name="bq_sb")
            bk_sb = spool.tile([P, KD], F32, tag="bk", name="bk_sb")
            bo_sb = spool.tile([P, KD], F32, tag="bo", name="bo_sb")
            for sb, dram in ((bq_sb, bq_d), (bk_sb, bk_d), (bo_sb, bo2_d)):
                nc.sync.dma_start(sb[:], dram.rearrange("(o p) -> p o", p=P))

            # ---- work queue of projection thunks (1 instruction each) ----
            accq = deque()

            def pump(n):
                for _ in range(n):
                    if not accq:
                        return
                    accq.popleft()()

            def qk_group(get_dst, wt, bias_sb, j, b, c):
                """8 MMs accumulating [128,512] + bias drain into dst."""
                st8 = {}

                def mm(k):
                    if "ps" not in st8:
                        st8["ps"] = accp.tile([P, 512], F32, tag="acc", name="acc")
                    nc.tensor.matmul(
                        st8["ps"][:],
                        wt[k][:, j * P : (j + 1) * P],
                        xt[k][:, b * TB + c * 512 : b * TB + (c + 1) * 512],
                        start=(k == 0),
                        stop=(k == KD - 1),
                    )

                def drain():
                    nc.scalar.activation(
                        get_dst()[:, c * 512 : (c + 1) * 512],
                        st8["ps"][:],
                        IDF,
                        bias=bias_sb[:, j : j + 1],
                    )

                return [lambda k=k: mm(k) for k in range(KD)] + [drain]

            def v_group(get_vt, b, mt, c):
                st8 = {}

                def mm(k):
                    if "ps" not in st8:
                        st8["ps"] = accp.tile([P, 512], F32, tag="acc", name="acc")
                    nc.tensor.matmul(
                        st8["ps"][:],
                        xt[k][:, (b * KD + mt) * P : (b * KD + mt + 1) * P],
                        wv[k][:, c * 512 : (c + 1) * 512],
                        start=(k == 0),
                        stop=(k == KD - 1),
                    )

                def drain():
                    nc.vector.tensor_copy(
                        get_vt()[:, c * 8 : (c + 1) * 8, 0:DH],
                        st8["ps"].rearrange("p (h d) -> p h d", d=DH),
                    )

                return [lambda k=k: mm(k) for k in range(KD)] + [drain]

            def o_group(ctxt, b, mo, c):
                st8 = {}

                def mm(k):
                    if "ps" not in st8:
                        st8["ps"] = accp.tile([P, 512], F32, tag="acc", name="acc")
                    nc.tensor.matmul(
                        st8["ps"][:],
                        wo[k][:, mo * P : (mo + 1) * P],
                        ctxt[k][:, c * 512 : (c + 1) * 512],
                        start=(k == 0),
                        stop=(k == KD - 1),
                    )

                def drain():
                    osb = opool.tile([P, 512], F32, tag="osb", name="osb")
                    nc.scalar.activation(osb[:], st8["ps"][:], IDF, bias=bo_sb[:, mo : mo + 1])
                    nc.sync.dma_start(
                        outt_d[
                            mo * P : (mo + 1) * P,
                            b * TB + c * 512 : b * TB + (c + 1) * 512,
                        ],
                        osb[:],
                    )

                return [lambda k=k: mm(k) for k in range(KD)] + [drain]

            # ---- tile registries (lazily created inside thunks) ----
            qk_tiles = {}  # (j, b, 'q'|'k') -> tile [P, TB]

            def get_qk(j, b, which):
                key = (j, b, which)
                if key not in qk_tiles:
                    qk_tiles[key] = qkpool.tile(
                        [P, TB], BF16, tag=f"{which}tj", name=f"{which}t{j}b{b}"
                    )
                return qk_tiles[key]

            v_tiles = {}  # (b, mt) -> tile [P, H, DH+1]

            def get_v(b, mt):
                key = (b, mt)
                if key not in v_tiles:
                    t = vpool.tile([P, H, DH + 1], BF16, tag=f"v{mt}", name=f"v{mt}b{b}")
                    v_tiles[key] = t
                    nc.vector.memset(t[:, :, DH : DH + 1], 1.0)
                return v_tiles[key]

            ctxt_tiles = {}  # b -> [8 tiles]

            def get_ctxt(b):
                if b not in ctxt_tiles:
                    ctxt_tiles[b] = [
                        cpool.tile([P, TB], BF16, tag=f"ctxt{m}", name=f"ctxt{m}b{b}")
                        for m in range(KD)
                    ]
                return ctxt_tiles[b]

            def enqueue_qk(j, b):
                for which, wt, bias in (("q", wq, bq_sb), ("k", wk, bk_sb)):
                    for c in range(2):
                        accq.extend(
                            qk_group(lambda j=j, b=b, w=which: get_qk(j, b, w), wt, bias, j, b, c)
                        )

            def enqueue_v(b, mts):
                for mt in mts:
                    for c in range(2):
                        accq.extend(v_group(lambda b=b, mt=mt: get_v(b, mt), b, mt, c))

            def enqueue_o(b, mos):
                ctxt = get_ctxt(b)
                for mo in mos:
                    for c in range(2):
                        accq.extend(o_group(ctxt, b, mo, c))

            # ---- attention slot for (j, b) ----
            def attention_slot(j, b, pump_n=4):
                qtj = get_qk(j, b, "q")
                ktj = get_qk(j, b, "k")
                ctxt = get_ctxt(b)
                h0, h1 = 2 * j, 2 * j + 1

                for cg in range(2):  # query half: 512 tokens per pass
                    q0, q1 = cg * 512, (cg + 1) * 512
                    pv0 = pvp.tile([P, 512], F32, tag="pv0", name=f"pv0_{j}_{b}_{cg}")
                    pv1 = pvp.tile([P, 512], F32, tag="pv1", name=f"pv1_{j}_{b}_{cg}")
                    pts = []

                    def pv_step(st, pv0=pv0, pv1=pv1, pts=pts):
                        pt = pts[st]
                        vt = get_v(b, st)
                        first, last = st == 0, st == KD - 1
                        nc.tensor.matmul(
                            pv0[0 : DH + 1, :], vt[:, h0, :], pt[:, 0:512],
                            start=first, stop=last,
                        )
                        nc.tensor.matmul(
                            pv1[0 : DH + 1, :], vt[:, h1, :], pt[:, 512:1024],
                            start=first, stop=last,
                        )

                    for st in range(KD):
                        pt = ptpool.tile([P, 1024], BF16, tag="pt", name=f"pt{st}")
                        pts.append(pt)
                        sc = scp.tile([P, 1024], F32, tag="sc", name="sc")
                        nc.tensor.matmul(
                            sc[:, 0:512],
                            ktj[0:DH, st * P : (st + 1) * P],
                            qtj[0:DH, q0:q1],
                            start=True, stop=True,
                        )
                        nc.tensor.matmul(
                            sc[:, 512:1024],
                            ktj[DH:P, st * P : (st + 1) * P],
                            qtj[DH:P, q0:q1],
                            start=True, stop=True,
                        )
                        nc.scalar.activation(
                            pt[:, 0:1024], sc[:, 0:1024], EXPF, scale=0.125
                        )
                        if st > 0:
                            pv_step(st - 1)
                        pump(pump_n)
                    pv_step(KD - 1)

                    # normalize h0 -> ctxt rows 0:64, h1 -> staged + DMA shift
                    for hh, pv in ((0, pv0), (1, pv1)):
                        rs = npool.tile([1, 512], F32, tag="rs", name="rs", bufs=1)
                        nc.vector.tensor_copy(rs[:], pv[DH : DH + 1, :])
                        rr = npool.tile([1, 512], F32, tag="rr", name="rr", bufs=1)
                        nc.vector.reciprocal_approx_fast(rr[:], rs[:])
                        rb = npool.tile([DH, 512], F32, tag="rb", name="rb")
                        nc.gpsimd.partition_broadcast(rb[:], rr[:])
                        if hh == 0:
                            nc.vector.tensor_tensor(
                                ctxt[j][0:DH, q0:q1], pv[0:DH, :], rb[:], MULT
                            )
                        else:
                            ch = npool.tile([DH, 512], BF16, tag="ch", name="ch")
                            nc.vector.tensor_tensor(ch[:], pv[0:DH, :], rb[:], MULT)
                            nc.sync.dma_start(ctxt[j][DH:P, q0:q1], ch[:])

            # ---- schedule ----
            # prologue: QK(j0, b0) + V(b0) drained
            enqueue_qk(0, 0)
            enqueue_v(0, range(KD))
            pump(len(accq))

            # batch 0 slots; V(b1) spread over slots 2-5, QK(j0,b1) at slot 6
            for j in range(KD):
                if j + 1 < KD:
                    enqueue_qk(j + 1, 0)
                if 2 <= j <= 5:
                    enqueue_v(1, range((j - 2) * 2, (j - 2) * 2 + 2))
                if j == 6:
                    enqueue_qk(0, 1)
                attention_slot(j, 0, pump_n=10 if j == 0 else 4)

            # batch 1 slots; o-proj of b0 spread two mo per early slot
            for j in range(KD):
                if j + 1 < KD:
                    enqueue_qk(j + 1, 1)
                if j < 4:
                    enqueue_o(0, [2 * j, 2 * j + 1])
                attention_slot(j, 1)
            enqueue_o(1, range(KD))
            pump(len(accq))

    nc.compile()
    return nc


def _get_nc():
    global _NC_CACHE
    if _NC_CACHE is None:
        _NC_CACHE = build_nc()
    return _NC_CACHE


def kernel(hidden_states, Wq, bq, Wk, bk, Wv, bv, Wo, bo):
    global LAST_RESULTS
    bf = ml_dtypes.bfloat16
    hs = np.asarray(hidden_states, np.float32)
    Wq = np.asarray(Wq, np.float32)
    Wk = np.asarray(Wk, np.float32)
    Wv = np.asarray(Wv, np.float32)
    Wo = np.asarray(Wo, np.float32)
    bq = np.asarray(bq, np.float32)
    bk = np.asarray(bk, np.float32)
    bv = np.asarray(bv, np.float32)
    bo = np.asarray(bo, np.float32)

    wqt = np.ascontiguousarray(Wq.T).astype(bf)
    wkt = np.ascontiguousarray(Wk.T).astype(bf)
    wvt = np.ascontiguousarray(Wv.T).astype(bf)
    wot = np.ascontiguousarray(Wo.T).astype(bf)
    bo2 = (bo + Wo @ bv).astype(np.float32)

    bpc = hs.shape[0] // NCORES  # batches per core
    in_maps = []
    for c in range(NCORES):
        xc = hs[c * bpc : (c + 1) * bpc].reshape(bpc * TB, D)
        in_maps.append(
            {
                "xt": np.ascontiguousarray(xc.T).astype(bf),
                "wqt": wqt,
                "wkt": wkt,
                "wvt": wvt,
                "wot": wot,
                "bq": bq,
                "bk": bk,
                "bo2": bo2,
            }
        )

    nc = _get_nc()
    res = run_bass_kernel_spmd(
        nc,
        in_maps,
        core_ids=list(range(NCORES)),
        trace=TRACE,
        **TRACE_KWARGS,
    )
    LAST_RESULTS = res

    out = np.empty((hs.shape[0], TB, D), np.float32)
    for c in range(NCORES):
        ot = res.results[c]["outt"]  # [D, T]
        for b in range(bpc):
            out[c * bpc + b] = ot[:, b * TB : (b + 1) * TB].T
    return out


# revision 18
# speedup vs baseline: 1.1663x; 1.0797x over previous
"""Multi-head attention forward (B=16, S=1024, d=1024, H=16, Dh=64) on 8
Trainium2 NeuronCores, data-parallel over batch (2 batches per core).

v2: row-tiled concurrent scores pairs (K=64 in rows 0-63 / 64-127), N=1024
cross-bank exp activations, deadline-ordered projection work queue to keep
the PE queue free of head-of-line stalls.

Device kernel (per core, bf16 matmuls, fp32 accumulate):
  inputs (host-prepped): XT [d, 2048] = hidden[2c:2c+2].reshape(2048,d).T,
  WqT/WkT/WvT = W.T [in, out], WoT = Wo.T [dv, o]  (all bf16),
  bq, bk [1024] f32, bo2 = bo + Wo @ bv  (bv folded: softmax rows sum to 1).

  Per (b, j=head-pair): QT/KT [128, 1024] (pair dims on partitions,
  h0 dims 0-63, h1 dims 64-127).
  scores pair (per st key-chunk, per c query-half):
    sc[:, 0:512]    = KT[0:64, st].T  @ QT[0:64, c]    (PE rows 0-63)
    sc[:, 512:1024] = KT[64:128, st].T @ QT[64:128, c] (PE rows 64-127, concurrent)
  exp over [128, 1024] psum (2 banks) -> pt[st] bf16.
  PV: ctx_aug[65, 1024] += [V_h | 1].T @ P_h  (row 64 = softmax denominator)
  normalize: ctx * bcast(1/denom); h1-half staged + DMA partition shift.
  outT[o, t] = WoT.T @ ctxT (+bo2).
"""

from collections import deque

import numpy as np
import ml_dtypes

import concourse.bass as bass
import concourse.mybir as mybir
import concourse.tile as tile
from concourse import bacc
from concourse.bass_utils import run_bass_kernel_spmd

P = 128
D = 1024
T = 2048  # tokens per core
TB = 1024  # tokens per batch (= S)
H = 16
DH = 64
KD = D // P  # 8 chunks of the d/dv/s dims
NB = T // TB  # batches per core
NCORES = 8

BF16 = mybir.dt.bfloat16
F32 = mybir.dt.float32
EXPF = mybir.ActivationFunctionType.Exp
IDF = mybir.ActivationFunctionType.Identity
MULT = mybir.AluOpType.mult

# test.py hooks
TRACE = False
TRACE_KWARGS = {}
LAST_RESULTS = None

_NC_CACHE = None


def build_nc():
    nc = bacc.Bacc("TRN2", target_bir_lowering=False, debug=False, num_devices=NCORES)

    xt_d = nc.dram_tensor("xt", [D, T], BF16, kind="ExternalInput")
    wqt_d = nc.dram_tensor("wqt", [D, D], BF16, kind="ExternalInput")
    wkt_d = nc.dram_tensor("wkt", [D, D], BF16, kind="ExternalInput")
    wvt_d = nc.dram_tensor("wvt", [D, D], BF16, kind="ExternalInput")
    wot_d = nc.dram_tensor("wot", [D, D], BF16, kind="ExternalInput")
    bq_d = nc.dram_tensor("bq", [D], F32, kind="ExternalInput")
    bk_d = nc.dram_tensor("bk", [D], F32, kind="ExternalInput")
    bo2_d = nc.dram_tensor("bo2", [D], F32, kind="ExternalInput")
    outt_d = nc.dram_tensor("outt", [D, T], F32, kind="ExternalOutput")

    with tile.TileContext(nc) as tc:
        from contextlib import ExitStack

        with ExitStack() as ctx:
            wpool = ctx.enter_context(tc.tile_pool(name="w", bufs=1))
            xpool = ctx.enter_context(tc.tile_pool(name="x", bufs=1))
            spool = ctx.enter_context(tc.tile_pool(name="small", bufs=1))
            qkpool = ctx.enter_context(tc.tile_pool(name="qk", bufs=2))
            vpool = ctx.enter_context(tc.tile_pool(name="v", bufs=2))
            ptpool = ctx.enter_context(tc.tile_pool(name="pt", bufs=6))
            cpool = ctx.enter_context(tc.tile_pool(name="ctx", bufs=2))
            npool = ctx.enter_context(tc.tile_pool(name="norm", bufs=2))
            opool = ctx.enter_context(tc.tile_pool(name="out", bufs=2))
            scp = ctx.enter_context(tc.tile_pool(name="scp", bufs=2, space="PSUM"))
            pvp = ctx.enter_context(tc.tile_pool(name="pvp", bufs=1, space="PSUM"))
            accp = ctx.enter_context(tc.tile_pool(name="accp", bufs=2, space="PSUM"))

            # ---- global loads ----
            xt = [xpool.tile([P, T], BF16, tag=f"xt{k}", name=f"xt{k}") for k in range(KD)]
            wq, wk, wv, wo = (
                [wpool.tile([P, D], BF16, tag=f"w{nm}{k}", name=f"w{nm}{k}") for k in range(KD)]
                for nm in "qkvo"
            )
            # DMA order = dependency order of the prologue: biases, xt,
            # wq/wk j0 columns (first qk groups), wv (v-proj), remainder.
            bq_sb = spool.tile([P, KD], F32, tag="bq", name="bq_sb")
            bk_sb = spool.tile([P, KD], F32, tag="bk", name="bk_sb")
            bo_sb = spool.tile([P, KD], F32, tag="bo", name="bo_sb")
            for sb, dram in ((bq_sb, bq_d), (bk_sb, bk_d), (bo_sb, bo2_d)):
                nc.sync.dma_start(sb[:], dram.rearrange("(o p) -> p o", p=P))
            for k in range(KD):
                nc.sync.dma_start(xt[k][:], xt_d[k * P : (k + 1) * P, :])
            for wt, wd in ((wq, wqt_d), (wk, wkt_d)):
                for k in range(KD):
                    nc.sync.dma_start(wt[k][:], wd[k * P : (k + 1) * P, :])
            for k in range(KD):
                nc.sync.dma_start(wv[k][:], wvt_d[k * P : (k + 1) * P, :])
            for k in range(KD):
                nc.sync.dma_start(wo[k][:], wot_d[k * P : (k + 1) * P, :])

            # ---- work queue of projection thunks (1 instruction each) ----
            accq = deque()

            def pump(n):
                for _ in range(n):
                    if not accq:
                        return
                    accq.popleft()()

            def qk_group(get_dst, wt, bias_sb, j, b, c):
                """8 MMs accumulating [128,512] + bias drain into dst."""
                st8 = {}

                def mm(k):
                    if "ps" not in st8:
                        st8["ps"] = accp.tile([P, 512], F32, tag="acc", name="acc")
                    nc.tensor.matmul(
                        st8["ps"][:],
                        wt[k][:, j * P : (j + 1) * P],
                        xt[k][:, b * TB + c * 512 : b * TB + (c + 1) * 512],
                        start=(k == 0),
                        stop=(k == KD - 1),
                    )

                def drain():
                    nc.scalar.activation(
                        get_dst()[:, c * 512 : (c + 1) * 512],
                        st8["ps"][:],
                        IDF,
                        bias=bias_sb[:, j : j + 1],
                    )

                return [lambda k=k: mm(k) for k in range(KD)] + [drain]

            def v_group(get_vt, b, mt, c):
                st8 = {}

                def mm(k):
                    if "ps" not in st8:
                        st8["ps"] = accp.tile([P, 512], F32, tag="acc", name="acc")
                    nc.tensor.matmul(
                        st8["ps"][:],
                        xt[k][:, (b * KD + mt) * P : (b * KD + mt + 1) * P],
                        wv[k][:, c * 512 : (c + 1) * 512],
                        start=(k == 0),
                        stop=(k == KD - 1),
                    )

                def drain():
                    nc.vector.tensor_copy(
                        get_vt()[:, c * 8 : (c + 1) * 8, 0:DH],
                        st8["ps"].rearrange("p (h d) -> p h d", d=DH),
                    )

                return [lambda k=k: mm(k) for k in range(KD)] + [drain]

            def o_group(ctxt, b, mo, c):
                st8 = {}

                def mm(k):
                    if "ps" not in st8:
                        st8["ps"] = accp.tile([P, 512], F32, tag="acc", name="acc")
                    nc.tensor.matmul(
                        st8["ps"][:],
                        wo[k][:, mo * P : (mo + 1) * P],
                        ctxt[k][:, c * 512 : (c + 1) * 512],
                        start=(k == 0),
                        stop=(k == KD - 1),
                    )

                def drain():
                    osb = opool.tile([P, 512], F32, tag="osb", name="osb")
                    nc.scalar.activation(osb[:], st8["ps"][:], IDF, bias=bo_sb[:, mo : mo + 1])
                    nc.sync.dma_start(
                        outt_d[
                            mo * P : (mo + 1) * P,
                            b * TB + c * 512 : b * TB + (c + 1) * 512,
                        ],
                        osb[:],
                    )

                return [lambda k=k: mm(k) for k in range(KD)] + [drain]

            # ---- tile registries (lazily created inside thunks) ----
            qk_tiles = {}  # (j, b, 'q'|'k') -> tile [P, TB]

            def get_qk(j, b, which):
                key = (j, b, which)
                if key not in qk_tiles:
                    qk_tiles[key] = qkpool.tile(
                        [P, TB], BF16, tag=f"{which}tj", name=f"{which}t{j}b{b}"
                    )
                return qk_tiles[key]

            v_tiles = {}  # (b, mt) -> tile [P, H, DH+1]

            def get_v(b, mt):
                key = (b, mt)
                if key not in v_tiles:
                    t = vpool.tile([P, H, DH + 1], BF16, tag=f"v{mt}", name=f"v{mt}b{b}")
                    v_tiles[key] = t
                    nc.vector.memset(t[:, :, DH : DH + 1], 1.0)
                return v_tiles[key]

            ctxt_tiles = {}  # b -> [8 tiles]

            def get_ctxt(b):
                if b not in ctxt_tiles:
                    ctxt_tiles[b] = [
                        cpool.tile([P, TB], BF16, tag=f"ctxt{m}", name=f"ctxt{m}b{b}")
                        for m in range(KD)
                    ]
                return ctxt_tiles[b]

            def enqueue_qk(j, b):
                for which, wt, bias in (("q", wq, bq_sb), ("k", wk, bk_sb)):
                    for c in range(2):
                        accq.extend(
                            qk_group(lambda j=j, b=b, w=which: get_qk(j, b, w), wt, bias, j, b, c)
                        )

            def enqueue_v(b, mts):
                for mt in mts:
                    for c in range(2):
                        accq.extend(v_group(lambda b=b, mt=mt: get_v(b, mt), b, mt, c))

            def enqueue_o(b, mos):
                ctxt = get_ctxt(b)
                for mo in mos:
                    for c in range(2):
                        accq.extend(o_group(ctxt, b, mo, c))

            # ---- attention slot for (j, b) ----
            def attention_slot(j, b, pump_n=4):
                qtj = get_qk(j, b, "q")
                ktj = get_qk(j, b, "k")
                ctxt = get_ctxt(b)
                h0, h1 = 2 * j, 2 * j + 1

                for cg in range(2):  # query half: 512 tokens per pass
                    q0, q1 = cg * 512, (cg + 1) * 512
                    pv0 = pvp.tile([P, 512], F32, tag="pv0", name=f"pv0_{j}_{b}_{cg}")
                    pv1 = pvp.tile([P, 512], F32, tag="pv1", name=f"pv1_{j}_{b}_{cg}")
                    pts = []

                    def pv_step(st, pv0=pv0, pv1=pv1, pts=pts):
                        pt = pts[st]
                        vt = get_v(b, st)
                        first, last = st == 0, st == KD - 1
                        nc.tensor.matmul(
                            pv0[0 : DH + 1, :], vt[:, h0, :], pt[:, 0:512],
                            start=first, stop=last,
                        )
                        nc.tensor.matmul(
                            pv1[0 : DH + 1, :], vt[:, h1, :], pt[:, 512:1024],
                            start=first, stop=last,
                        )

                    for st in range(KD):
                        pt = ptpool.tile([P, 1024], BF16, tag="pt", name=f"pt{st}")
                        pts.append(pt)
                        sc = scp.tile([P, 1024], F32, tag="sc", name="sc")
                        nc.tensor.matmul(
                            sc[:, 0:512],
                            ktj[0:DH, st * P : (st + 1) * P],
                            qtj[0:DH, q0:q1],
                            start=True, stop=True,
                        )
                        nc.tensor.matmul(
                            sc[:, 512:1024],
                            ktj[DH:P, st * P : (st + 1) * P],
                            qtj[DH:P, q0:q1],
                            start=True, stop=True,
                        )
                        nc.scalar.activation(
                            pt[:, 0:1024], sc[:, 0:1024], EXPF, scale=0.125
                        )
                        if st > 0:
                            pv_step(st - 1)
                        pump(pump_n)
                    pv_step(KD - 1)

                    # normalize: stage pv to SBUF in one copy (frees the psum
                    # bank fast), then recip/bcast/mult from the staging tile.
                    for hh, pv in ((0, pv0), (1, pv1)):
                        stg = npool.tile([DH, 512], F32, tag="stg", name="stg")
                        nc.vector.tensor_copy(stg[:], pv[0:DH, :])
                        rs = npool.tile([1, 512], F32, tag="rs", name="rs", bufs=1)
                        nc.vector.tensor_copy(rs[:], pv[DH : DH + 1, :])
                        rr = npool.tile([1, 512], F32, tag="rr", name="rr", bufs=1)
                        nc.vector.reciprocal_approx_fast(rr[:], rs[:])
                        rb = npool.tile([DH, 512], F32, tag="rb", name="rb")
                        nc.gpsimd.partition_broadcast(rb[:], rr[:])
                        if hh == 0:
                            nc.vector.tensor_tensor(
                                ctxt[j][0:DH, q0:q1], stg[:], rb[:], MULT
                            )
                        else:
                            ch = npool.tile([DH, 512], BF16, tag="ch", name="ch")
                            nc.vector.tensor_tensor(ch[:], stg[:], rb[:], MULT)
                            nc.sync.dma_start(ctxt[j][DH:P, q0:q1], ch[:])

            # ---- schedule ----
            # prologue: QK(j0, b0) + V(b0) drained
            enqueue_qk(0, 0)
            enqueue_v(0, range(KD))
            pump(len(accq))

            # batch 0 slots; V(b1) spread over slots 2-5, QK(j0,b1) at slot 6
            for j in range(KD):
                if j + 1 < KD:
                    enqueue_qk(j + 1, 0)
                if 2 <= j <= 5:
                    enqueue_v(1, range((j - 2) * 2, (j - 2) * 2 + 2))
                if j == 6:
                    enqueue_qk(0, 1)
                attention_slot(j, 0, pump_n=10 if j == 0 else 4)

            # batch 1 slots; o-proj of b0 spread two mo per early slot
            for j in range(KD):
                if j + 1 < KD:
                    enqueue_qk(j + 1, 1)
                if j < 4:
                    enqueue_o(0, [2 * j, 2 * j + 1])
                attention_slot(j, 1)
            enqueue_o(1, range(KD))
            pump(len(accq))

    nc.compile()
    return nc


def _get_nc():
    global _NC_CACHE
    if _NC_CACHE is None:
        _NC_CACHE = build_nc()
    return _NC_CACHE


def kernel(hidden_states, Wq, bq, Wk, bk, Wv, bv, Wo, bo):
    global LAST_RESULTS
    bf = ml_dtypes.bfloat16
    hs = np.asarray(hidden_states, np.float32)
    Wq = np.asarray(Wq, np.float32)
    Wk = np.asarray(Wk, np.float32)
    Wv = np.asarray(Wv, np.float32)
    Wo = np.asarray(Wo, np.float32)
    bq = np.asarray(bq, np.float32)
    bk = np.asarray(bk, np.float32)
    bv = np.asarray(bv, np.float32)
    bo = np.asarray(bo, np.float32)

    wqt = np.ascontiguousarray(Wq.T).astype(bf)
    wkt = np.ascontiguousarray(Wk.T).astype(bf)
    wvt = np.ascontiguousarray(Wv.T).astype(bf)
    wot = np.ascontiguousarray(Wo.T).astype(bf)
    bo2 = (bo + Wo @ bv).astype(np.float32)

    bpc = hs.shape[0] // NCORES  # batches per core
    in_maps = []
    for c in range(NCORES):
        xc = hs[c * bpc : (c + 1) * bpc].reshape(bpc * TB, D)
        in_maps.append(
            {
                "xt": np.ascontiguousarray(xc.T).astype(bf),
                "wqt": wqt,
                "wkt": wkt,
                "wvt": wvt,
                "wot": wot,
                "bq": bq,
                "bk": bk,
                "bo2": bo2,
            }
        )

    nc = _get_nc()
    res = run_bass_kernel_spmd(
        nc,
        in_maps,
        core_ids=list(range(NCORES)),
        trace=TRACE,
        **TRACE_KWARGS,
    )
    LAST_RESULTS = res

    out = np.empty((hs.shape[0], TB, D), np.float32)
    for c in range(NCORES):
        ot = res.results[c]["outt"]  # [D, T]
        for b in range(bpc):
            out[c * bpc + b] = ot[:, b * TB : (b + 1) * TB].T
    return out
